# revision 7
# baseline (speedup 1.0000x reference)
"""VQ codebook-lookup kernel for Trainium2 (8 NeuronCores, data-parallel over tokens).

For each of B*T=16384 tokens (D=1024) find the nearest of K=4096 codebook rows
under squared-L2 distance and emit the gathered codebook row (the forward value
of the straight-through estimator is exactly embedding[argmin]).

Distance argmin trick: argmin_k ||x-e_k||^2 = argmax_k (2*x.e_k - ||e_k||^2).
The 2*x.e_k scores are computed on the PE array with an fp16 hi/lo split
(XH*EH + XH*EL + XL*EH, each an fp16 matmul with fp32 PSUM accumulation),
which keeps the score error ~1e-4 -- far below the minimum top-2 distance gap
-- so the argmin matches an exact fp32 computation. The ||e_k||^2 bias is
subtracted on the Vector engine, which also finds the per-token max and its
index (nc.vector.max / max_index). The winning codebook rows are fetched with
an indirect (gather) DMA from DRAM at full fp32 precision.

Sharding: tokens are split 16384/8 = 2048 per core; the codebook is replicated.
"""

import sys

import numpy as np

try:
    import concourse  # noqa: F401
except ImportError:
    sys.path.append("/opt/trn_rl_repo")

B, T, D = 8, 2048, 1024
K = 4096
P = 128
N_CORES = 8
TOK_PER_CORE = B * T // N_CORES    # 2048
N_TT = TOK_PER_CORE // P           # 16 token tiles per core
N_DC = D // P                      # 8 contraction chunks
CC = 512                           # codes per PSUM bank
N_CC = K // CC                     # 8 code chunks

TRACE = False
LAST_RESULT = None

_PROG_CACHE = {}


def _build_program(n_tt, repeat=1):
    import concourse.bass as bass
    import concourse.tile as tile
    from concourse import bacc, mybir

    f16 = mybir.dt.float16
    f32 = mybir.dt.float32

    nc = bacc.Bacc("TRN2", debug=False, num_devices=N_CORES)

    xt_d = nc.dram_tensor("xt", [n_tt, P, 2, N_DC, P], f16, kind="ExternalInput").ap()
    et_d = nc.dram_tensor("et", [2, N_DC, P, K], f16, kind="ExternalInput").ap()
    e2_d = nc.dram_tensor("e2r", [P, K], f32, kind="ExternalInput").ap()
    emb_d = nc.dram_tensor("emb", [K, D], f32, kind="ExternalInput").ap()
    out_d = nc.dram_tensor("out", [n_tt * P, D], f32, kind="ExternalOutput").ap()

    with tile.TileContext(nc) as tc:
        with (
            tc.tile_pool(name="const", bufs=1) as const_pool,
            tc.tile_pool(name="xtp", bufs=2) as xt_pool,
            tc.tile_pool(name="distp", bufs=1) as dist_pool,
            tc.tile_pool(name="smallp", bufs=4) as small_pool,
            tc.tile_pool(name="gathp", bufs=2) as gath_pool,
            tc.tile_pool(name="psump", bufs=2, space="PSUM") as psum_pool,
        ):
            # resident transposed codebook (hi/lo fp16) + ||e||^2 bias
            et_sb = const_pool.tile([P, 2, N_DC, K], f16)
            for hl in range(2):
                for dc in range(N_DC):
                    nc.sync.dma_start(out=et_sb[:, hl, dc, :], in_=et_d[hl, dc])
            e2_sb = const_pool.tile([P, K], f32)
            nc.sync.dma_start(out=e2_sb[:], in_=e2_d)

            terms = [(0, 0), (0, 1), (1, 0)]  # (x hi/lo, e hi/lo) matmul terms
            half_cc = N_CC // 2
            for tt in [t for _ in range(repeat) for t in range(n_tt)]:
                xt_sb = xt_pool.tile([P, 2, N_DC, P], f16)
                nc.sync.dma_start(out=xt_sb[:], in_=xt_d[tt])

                dist_sb = dist_pool.tile([P, K], f32)
                for half in range(2):
                    psh = psum_pool.tile([P, half_cc, CC], f32, name="psh")
                    for dc in range(N_DC):
                        for ti, (xh, eh) in enumerate(terms):
                            first = dc == 0 and ti == 0
                            last = dc == N_DC - 1 and ti == len(terms) - 1
                            for c4 in range(half_cc):
                                cc = half * half_cc + c4
                                nc.tensor.matmul(
                                    psh[:, c4, :],
                                    lhsT=xt_sb[:, xh, dc, :],
                                    rhs=et_sb[:, eh, dc, cc * CC:(cc + 1) * CC],
                                    start=first,
                                    stop=last,
                                )
                    for c4 in range(half_cc):
                        cc = half * half_cc + c4
                        nc.vector.tensor_sub(
                            dist_sb[:, cc * CC:(cc + 1) * CC],
                            psh[:, c4, :],
                            e2_sb[:, cc * CC:(cc + 1) * CC],
                        )

                mx = small_pool.tile([P, 8], f32)
                midx = small_pool.tile([P, 8], mybir.dt.uint32)
                nc.vector.max(out=mx[:], in_=dist_sb[:])
                nc.vector.max_index(out=midx[:], in_max=mx[:], in_values=dist_sb[:])

                gath = gath_pool.tile([P, D], f32)
                nc.gpsimd.indirect_dma_start(
                    out=gath[:],
                    out_offset=None,
                    in_=emb_d,
                    in_offset=bass.IndirectOffsetOnAxis(ap=midx[:, :1], axis=0),
                )
                nc.sync.dma_start(out=out_d[tt * P:(tt + 1) * P, :], in_=gath[:])

    nc.compile()
    return nc


def _split16(a):
    hi = a.astype(np.float16)
    lo = (a - hi.astype(np.float32)).astype(np.float16)
    return hi, lo


def _host_prep(x, embedding, n_cores=N_CORES, n_tt=N_TT):
    x_flat = np.ascontiguousarray(np.asarray(x, dtype=np.float32)).reshape(B * T, D)
    E = np.ascontiguousarray(np.asarray(embedding, dtype=np.float32))

    eh, el = _split16(E)
    se = np.stack([eh, el])                       # [2, K, D]
    et = np.ascontiguousarray(
        se.reshape(2, K, N_DC, P).transpose(0, 2, 3, 1)  # [2, dc, p, K]
    )
    e2 = (E.astype(np.float64) ** 2).sum(1).astype(np.float32)
    e2r = np.ascontiguousarray(np.broadcast_to(e2, (P, K)))

    tok = n_tt * P
    in_maps = []
    for c in range(n_cores):
        xs = x_flat[c * TOK_PER_CORE: c * TOK_PER_CORE + tok]
        xh, xl = _split16(2.0 * xs)
        s = np.stack([xh, xl])                    # [2, tok, D]
        s = s.reshape(2, n_tt, P, N_DC, P)        # [hl, tt, t, dc, p]
        xt = np.ascontiguousarray(s.transpose(1, 4, 0, 3, 2))  # [tt, p, hl, dc, t]
        in_maps.append({"xt": xt, "et": et, "e2r": e2r, "emb": E})
    return in_maps


def _run(in_maps, n_tt=N_TT, repeat=1):
    from concourse import bass_utils

    key = (n_tt, repeat)
    if key not in _PROG_CACHE:
        _PROG_CACHE[key] = _build_program(n_tt, repeat)
    nc = _PROG_CACHE[key]
    return bass_utils.run_bass_kernel_spmd(
        nc, in_maps, core_ids=list(range(N_CORES)), trace=TRACE
    )


def kernel(x, embedding):
    global LAST_RESULT
    in_maps = _host_prep(x, embedding)
    res = _run(in_maps)
    LAST_RESULT = res
    out = np.concatenate([r["out"] for r in res.results], axis=0)
    return out.reshape(B, T, D)


# revision 14
# speedup vs baseline: 372.2354x; 372.2354x over previous
"""VQ codebook-lookup kernel for Trainium2 (8 NeuronCores, data-parallel over tokens).

For each of B*T=16384 tokens (D=1024) find the nearest of K=4096 codebook rows
under squared-L2 distance and emit the gathered codebook row (the forward value
of the straight-through estimator is exactly embedding[argmin]).

Distance argmin trick: argmin_k ||x-e_k||^2 = argmax_k (2*x.e_k - ||e_k||^2).
The 2*x.e_k scores are computed on the PE array with an fp16 hi/lo split
(XH*EH + XH*EL + XL*EH, each an fp16 matmul with fp32 PSUM accumulation),
which keeps the score error ~1e-4 -- far below the minimum top-2 distance gap
-- so the argmin matches an exact fp32 computation. The ||e_k||^2 bias is
subtracted on the Vector engine, which also finds the per-token max and its
index (nc.vector.max / max_index). The winning codebook rows are fetched with
an indirect (gather) DMA from DRAM at full fp32 precision.

Sharding: tokens are split 16384/8 = 2048 per core; the codebook is replicated.
"""

import sys

import numpy as np

try:
    import concourse  # noqa: F401
except ImportError:
    sys.path.append("/opt/trn_rl_repo")

B, T, D = 8, 2048, 1024
K = 4096
P = 128
N_CORES = 8
TOK_PER_CORE = B * T // N_CORES    # 2048
N_TT = TOK_PER_CORE // P           # 16 token tiles per core
N_DC = D // P                      # 8 contraction chunks
CC = 512                           # codes per PSUM bank
N_CC = K // CC                     # 8 code chunks

TRACE = False
LAST_RESULT = None

_PROG_CACHE = {}


def _build_program(n_tt, repeat=1, loop=None, mm_dtype="float16"):
    import concourse.bass as bass
    import concourse.tile as tile
    from concourse import bacc, mybir

    f16 = getattr(mybir.dt, mm_dtype)
    f32 = mybir.dt.float32

    nc = bacc.Bacc("TRN2", debug=False, num_devices=N_CORES)

    xt_d = nc.dram_tensor("xt", [n_tt, P, 2, N_DC, P], f16, kind="ExternalInput").ap()
    et_d = nc.dram_tensor("et", [2, N_DC, P, K], f16, kind="ExternalInput").ap()
    e2_d = nc.dram_tensor("e2r", [P, K], f32, kind="ExternalInput").ap()
    emb_d = nc.dram_tensor("emb", [K, D], f32, kind="ExternalInput").ap()
    out_d = nc.dram_tensor("out", [n_tt * P, D], f32, kind="ExternalOutput").ap()

    with tile.TileContext(nc) as tc:
        with (
            tc.tile_pool(name="const", bufs=1) as const_pool,
            tc.tile_pool(name="xtp", bufs=2) as xt_pool,
            tc.tile_pool(name="distp", bufs=1) as dist_pool,
            tc.tile_pool(name="smallp", bufs=4) as small_pool,
            tc.tile_pool(name="gathp", bufs=2) as gath_pool,
            tc.tile_pool(name="psump", bufs=2, space="PSUM") as psum_pool,
        ):
            # resident transposed codebook (hi/lo) + ||e||^2 bias; one tile per
            # (hi/lo, d-chunk) so compute can start as soon as its chunk lands
            et_t = {}
            for hl in range(2):
                for dc in range(N_DC):
                    et_t[hl, dc] = const_pool.tile([P, K], f16, name=f"et_{hl}_{dc}")
                    nc.sync.dma_start(out=et_t[hl, dc][:], in_=et_d[hl, dc])
            e2_sb = const_pool.tile([P, K], f32)
            nc.sync.dma_start(out=e2_sb[:], in_=e2_d)

            terms = [(0, 0), (0, 1), (1, 0)]  # (x hi/lo, e hi/lo) matmul terms
            half_cc = N_CC // 2

            def body():
                for tt in [t for _ in range(repeat) for t in range(n_tt)]:
                    xt_sb = xt_pool.tile([P, 2, N_DC, P], f16, name="xt_sb")
                    nc.sync.dma_start(out=xt_sb[:], in_=xt_d[tt])

                    dist_sb = dist_pool.tile([P, K], f32, name="dist_sb")
                    for half in range(2):
                        psh = psum_pool.tile([P, half_cc, CC], f32, name="psh")
                        for dc in range(N_DC):
                            for ti, (xh, eh) in enumerate(terms):
                                first = dc == 0 and ti == 0
                                last = dc == N_DC - 1 and ti == len(terms) - 1
                                for c4 in range(half_cc):
                                    cc = half * half_cc + c4
                                    nc.tensor.matmul(
                                        psh[:, c4, :],
                                        lhsT=xt_sb[:, xh, dc, :],
                                        rhs=et_t[eh, dc][:, cc * CC:(cc + 1) * CC],
                                        start=first,
                                        stop=last,
                                    )
                        for c4 in range(half_cc):
                            cc = half * half_cc + c4
                            nc.vector.tensor_sub(
                                dist_sb[:, cc * CC:(cc + 1) * CC],
                                psh[:, c4, :],
                                e2_sb[:, cc * CC:(cc + 1) * CC],
                            )

                    mx = small_pool.tile([P, 8], f32, name="mx")
                    midx = small_pool.tile([P, 8], mybir.dt.uint32, name="midx")
                    nc.vector.max(out=mx[:], in_=dist_sb[:])
                    nc.vector.max_index(out=midx[:], in_max=mx[:], in_values=dist_sb[:])

                    gath = gath_pool.tile([P, D], f32, name="gath")
                    nc.gpsimd.indirect_dma_start(
                        out=gath[:],
                        out_offset=None,
                        in_=emb_d,
                        in_offset=bass.IndirectOffsetOnAxis(ap=midx[:, :1], axis=0),
                    )
                    nc.sync.dma_start(out=out_d[tt * P:(tt + 1) * P, :], in_=gath[:])

            if loop is not None:
                with tc.For_i(0, loop, 1):
                    body()
            else:
                body()

    nc.compile()
    return nc


def _np16(mm_dtype):
    if mm_dtype == "float16":
        return np.float16
    import ml_dtypes

    return ml_dtypes.bfloat16


def _split16(a, dt16=np.float16):
    hi = a.astype(dt16)
    lo = (a - hi.astype(np.float32)).astype(dt16)
    return hi, lo


def _host_prep(x, embedding, n_cores=N_CORES, n_tt=N_TT, mm_dtype="float16"):
    dt16 = _np16(mm_dtype)
    x_flat = np.ascontiguousarray(np.asarray(x, dtype=np.float32)).reshape(B * T, D)
    E = np.ascontiguousarray(np.asarray(embedding, dtype=np.float32))

    eh, el = _split16(E, dt16)
    se = np.stack([eh, el])                       # [2, K, D]
    et = np.ascontiguousarray(
        se.reshape(2, K, N_DC, P).transpose(0, 2, 3, 1)  # [2, dc, p, K]
    )
    e2 = (E.astype(np.float64) ** 2).sum(1).astype(np.float32)
    e2r = np.ascontiguousarray(np.broadcast_to(e2, (P, K)))

    tok = n_tt * P
    in_maps = []
    for c in range(n_cores):
        xs = x_flat[c * TOK_PER_CORE: c * TOK_PER_CORE + tok]
        xh, xl = _split16(2.0 * xs, dt16)
        s = np.stack([xh, xl])                    # [2, tok, D]
        s = s.reshape(2, n_tt, P, N_DC, P)        # [hl, tt, t, dc, p]
        xt = np.ascontiguousarray(s.transpose(1, 4, 0, 3, 2))  # [tt, p, hl, dc, t]
        in_maps.append({"xt": xt, "et": et, "e2r": e2r, "emb": E})
    return in_maps


def _run(in_maps, n_tt=N_TT, repeat=1):
    from concourse import bass_utils

    key = (n_tt, repeat)
    if key not in _PROG_CACHE:
        _PROG_CACHE[key] = _build_program(n_tt, repeat)
    nc = _PROG_CACHE[key]
    return bass_utils.run_bass_kernel_spmd(
        nc, in_maps, core_ids=list(range(N_CORES)), trace=TRACE
    )


def kernel(x, embedding):
    global LAST_RESULT
    in_maps = _host_prep(x, embedding)
    res = _run(in_maps)
    LAST_RESULT = res
    out = np.concatenate([r["out"] for r in res.results], axis=0)
    return out.reshape(B, T, D)
